# revision 12
# baseline (speedup 1.0000x reference)
"""Causal self-attention (GQA 16q/4kv, rms_norm + rope + q_gain) on 8 trn2 cores.

Sharding: tensor-parallel over heads. Core c owns q-heads {2c, 2c+1} and kv-head
c//2. Each core computes its heads' attention output y_h^T [128, S] and a
partial projection  partial_c^T = Wproj[:, cols_c].T^T-contracted  [1024, S];
the host sums the 8 partials and transposes back to [1, S, 1024].

Layout strategy (per core, everything "d-major" = feature dim on partitions):
  Qt  [128, S] f32  rows = 2 heads x 64 dims  (rms-normed, roped, gained)
  Kt2 [128, S] f32  kv head duplicated to both partition halves (row-packed St)
  V   [128, 80*KB] bf16  seq-major tiles [128, 65] (col 64 = ones -> softmax sums)
  St block = Kt_blk.T @ Qt_chunk -> [128 k, 512 q] psum (2 heads row-packed)
  exp on ACT (scale=1/8) -> bf16, causal mask multiply on diagonal blocks
  AV: y^T[65, 512] += Vones_blk.T @ St_exp  (row 64 = sum of exp = softmax denom)
  normalize via DVE reciprocal + gpsimd partition_broadcast
  proj: partial^T[o-tile, q] = WprojT_tile.T @ y^T  -> DMA out
"""

import sys

for _p in ("/opt/trn_rl_repo",):
    if _p not in sys.path:
        sys.path.insert(0, _p)

import numpy as np
from contextlib import ExitStack

import concourse.bass as bass
import concourse.tile as tile
from concourse import mybir
from concourse.bass_utils import run_bass_kernel_spmd

F32 = mybir.dt.float32
F32R = mybir.dt.float32r
BF16 = mybir.dt.bfloat16
AF = mybir.ActivationFunctionType

DIM = 1024
NUM_HEADS = 16
NUM_KV_HEADS = 4
HD = 64
ROPE_BASE = 10000.0
RMS_EPS = 1.1920929e-07
N_CORES = 8

SCALE = 1.0 / 8.0  # 1/sqrt(64)

# Head-dim permutation: rope partner (d, d+32) placed in the same
# 32-partition quadrant so the rope half-swap is one stream_shuffle.
PERM64 = np.array([*range(0, 16), *range(32, 48), *range(16, 32), *range(48, 64)])
SWAP_MASK = [*range(16, 32), *range(0, 16)]  # swap halves within each quadrant


def build_nc(S: int, split: bool = True, iters: int = 1) -> bass.Bass:
    """Build the per-core SPMD Bass program for sequence length S (mult of 512)."""
    assert S % 512 == 0
    NCH = S // 512          # 512-wide seq chunks
    NKB = S // 128          # 128-wide k blocks

    nc = bass.Bass("TRN2", debug=False)

    xt_d = nc.declare_dram_parameter("xt", [DIM, S], F32R, isOutput=False)
    wqkv_d = nc.declare_dram_parameter("wqkv_t", [DIM, 256], F32R, isOutput=False)
    wproj_d = nc.declare_dram_parameter("wproj_t", [128, DIM], F32R, isOutput=False)
    cos_d = nc.declare_dram_parameter("cos4", [128, S], F32, isOutput=False)
    sin_d = nc.declare_dram_parameter("sin4s", [128, S], F32, isOutput=False)
    gain_d = nc.declare_dram_parameter("gain", [128, 1], F32, isOutput=False)
    out_d = nc.declare_dram_parameter("out", [DIM, S], F32, isOutput=True)
    vt_dram = nc.dram_tensor("vt_scratch", [64, S], BF16)

    with tile.TileContext(nc) as tc, ExitStack() as ctx:
        res = ctx.enter_context(tc.tile_pool(name="res", bufs=1))
        xtp = ctx.enter_context(tc.tile_pool(name="xtp", bufs=2))
        ropep = ctx.enter_context(tc.tile_pool(name="ropep", bufs=2))
        rowp = ctx.enter_context(tc.tile_pool(name="rowp", bufs=4))
        vtp = ctx.enter_context(tc.tile_pool(name="vtp", bufs=2))
        sep = ctx.enter_context(tc.tile_pool(name="sep", bufs=3))
        outp = ctx.enter_context(tc.tile_pool(name="outp", bufs=1))
        # PSUM budget (8 banks): stp/ms ring 2x2 + qkv 1x2 + y/po ring 2x1
        pst = ctx.enter_context(tc.tile_pool(name="pst", bufs=2, space="PSUM"))
        pqk = ctx.enter_context(tc.tile_pool(name="pqk", bufs=1, space="PSUM"))
        pyp = ctx.enter_context(tc.tile_pool(name="pyp", bufs=2, space="PSUM"))

        # ---- resident tiles ----
        qt_sb = res.tile([128, S], F32R, tag="qt", name="qt_sb")
        kt2_sb = res.tile([128, S], F32R, tag="kt2", name="kt2_sb")
        yt_sb = res.tile([128, S], F32R, tag="yt", name="yt_sb")
        v_sb = res.tile([128, NKB, 80], BF16, tag="v", name="v_sb")
        wqkv_sb = res.tile([128, 8, 256], F32R, tag="wqkv", name="wqkv_sb")
        wproj_sb = res.tile([128, DIM], F32R, tag="wproj", name="wproj_sb")
        cos_sb = res.tile([128, S], F32, tag="cos", name="cos_sb")
        sin_sb = res.tile([128, S], F32, tag="sin", name="sin_sb")
        gain_sb = res.tile([128, 1], F32, tag="gain", name="gain_sb")
        ones_m = res.tile([128, 64], BF16, tag="onesm", name="ones_m")
        ones_mk = res.tile([64, 128], BF16, tag="onesmk", name="ones_mk")
        ones_r = res.tile([1, 64], F32, tag="onesr", name="ones_r")
        ones_sb = res.tile([128, 1], F32, tag="ones", name="ones_sb")
        const_sb = res.tile([128, 2], F32, tag="const", name="const_sb")
        fbq = res.tile([128, 512], F32, tag="fbq", name="fbq")
        fbk = res.tile([64, 512], F32, tag="fbk", name="fbk")
        rr0 = res.tile([64, 512], F32, tag="rb0", name="rr0")
        rr1 = res.tile([64, 512], F32, tag="rb1", name="rr1")

        # ---- one-time setup ----
        for dt in range(8):
            nc.sync.dma_start(out=wqkv_sb[:, dt, :], in_=wqkv_d[dt * 128:(dt + 1) * 128, :])
        nc.sync.dma_start(out=wproj_sb[:], in_=wproj_d[:])
        nc.sync.dma_start(out=cos_sb[:], in_=cos_d[:])
        nc.sync.dma_start(out=sin_sb[:], in_=sin_d[:])
        nc.sync.dma_start(out=gain_sb[:], in_=gain_d[:])
        nc.vector.memset(ones_sb[:], 1.0)
        nc.vector.memset(ones_m[:], 1.0)
        nc.vector.memset(ones_mk[:], 1.0)
        nc.vector.memset(ones_r[:], 1.0)
        nc.vector.memset(const_sb[:, 0:1], 0.0)
        nc.vector.memset(const_sb[:, 1:2], RMS_EPS)
        nc.vector.memset(fbq[:], 0.0)
        nc.vector.memset(fbk[:], 0.0)
        nc.vector.memset(rr0[:], 0.0)
        nc.vector.memset(rr1[:], 0.0)
        zb128 = const_sb[:, 0:1]          # zero bias, 128 partitions
        zb64 = const_sb[0:64, 0:1]
        epsb = const_sb[0:1, 1:2]         # rms eps bias, 1 partition
        # ones column of every V tile (col 64 of each 80-wide block)
        nc.vector.memset(v_sb[:, :, 64:65], 1.0)

        def _chunks():
            def qkv_phase(c):
                s0 = c * 512
                sl = slice(s0, s0 + 512)

                # ---------- QKV projection (d-major): contraction over DIM ----------
                xtile = xtp.tile([128, 8, 512], F32R, tag="xt", name="xtile")
                xt_src = bass.AP(xt_d[:].tensor, s0, [[S, 128], [128 * S, 8], [1, 512]])
                nc.sync.dma_start(out=xtile[:], in_=xt_src)
                qkv_ps = pqk.tile([128, 1024], F32, tag="qkv", name="qkv_ps")
                for dt in range(8):
                    nc.tensor.matmul(qkv_ps[:, 0:512], lhsT=wqkv_sb[:, dt, 0:128],
                                     rhs=xtile[:, dt, :], start=(dt == 0), stop=(dt == 7))
                for dt in range(8):
                    nc.tensor.matmul(qkv_ps[:, 512:1024], lhsT=wqkv_sb[:, dt, 128:256],
                                     rhs=xtile[:, dt, :], start=(dt == 0), stop=(dt == 7))

                # ---------- rms_norm factors (all ACT funcs stay in the
                # natural_log_exp table set: square/ln/exp -> no table loads)
                sq = ropep.tile([128, 1024], BF16, tag="sqq", name="sq")
                nc.scalar.activation(sq[:], qkv_ps[:], AF.Square, bias=zb128)

                ms = pst.tile([128, 1024], F32, tag="st", name="ms")
                nc.tensor.matmul(ms[0:64, 0:512], lhsT=ones_m[0:64, :], rhs=sq[0:64, 0:512],
                                 start=True, stop=True)
                nc.tensor.matmul(ms[64:128, 0:512], lhsT=ones_m[64:128, :], rhs=sq[64:128, 0:512],
                                 start=True, stop=True, tile_position=(64, 64))
                nc.tensor.matmul(ms[:, 512:1024], lhsT=ones_mk[:], rhs=sq[0:64, 512:1024],
                                 start=True, stop=True)

                # rsqrt via ln+exp (exp table); gain folded into exp bias
                fblog = ropep.tile([128, 1024], F32, tag="flog", name="fblog")
                nc.scalar.activation(fblog[:], ms[:], AF.Ln, bias=const_sb[:, 1:2], scale=1.0 / HD)
                nc.scalar.activation(fbq[:], fblog[:, 0:512], AF.Exp,
                                     bias=gain_sb[:, 0:1], scale=-0.5)
                nc.scalar.activation(fbk[:], fblog[0:64, 512:1024], AF.Exp,
                                     bias=zb64, scale=-0.5)

                # ---------- rope + norm, q (2 heads, 128 partitions) ----------
                # head dims host-permuted so the rope partner lives in the
                # same 32-partition quadrant -> one stream_shuffle
                qsw = ropep.tile([128, 512], F32, tag="qsw", name="qsw")
                nc.vector.stream_shuffle(qsw[:], qkv_ps[:, 0:512], SWAP_MASK)
                t1 = ropep.tile([128, 512], F32, tag="t1", name="t1")
                nc.vector.tensor_mul(t1[:], qkv_ps[:, 0:512], cos_sb[:, sl])
                t2 = ropep.tile([128, 512], F32, tag="t2", name="t2")
                nc.vector.tensor_mul(t2[:], qsw[:], sin_sb[:, sl])
                nc.vector.tensor_add(t1[:], t1[:], t2[:])
                nc.vector.tensor_mul(qt_sb[:, sl], t1[:], fbq[:])

                # ---------- rope + norm, k (1 kv head, 64 partitions) ----------
                ksw = ropep.tile([64, 512], F32, tag="ksw", name="ksw")
                nc.vector.stream_shuffle(ksw[:], qkv_ps[0:64, 512:1024], SWAP_MASK)
                kt1 = ropep.tile([64, 512], F32, tag="kt1", name="kt1")
                nc.vector.tensor_mul(kt1[:], qkv_ps[0:64, 512:1024], cos_sb[0:64, sl])
                kt2t = ropep.tile([64, 512], F32, tag="kt2t", name="kt2t")
                nc.vector.tensor_mul(kt2t[:], ksw[:], sin_sb[0:64, sl])
                nc.vector.tensor_add(kt1[:], kt1[:], kt2t[:])
                nc.vector.tensor_mul(kt2_sb[0:64, sl], kt1[:], fbk[:])
                # duplicate kv head to partitions 64..127 (quadrant-aligned moves)
                nc.vector.tensor_copy(kt2_sb[64:96, sl], kt2_sb[0:32, sl])
                nc.vector.tensor_copy(kt2_sb[96:128, sl], kt2_sb[32:64, sl])

                # ---------- V: bf16, transpose to seq-major via HBM bounce ----------
                vtt = vtp.tile([64, 512], BF16, tag="vt", name="vtt")
                nc.vector.tensor_copy(vtt[:], qkv_ps[64:128, 512:1024])
                nc.sync.dma_start(out=vt_dram[:, sl], in_=vtt[:])
                for j in range(4):
                    kb = 4 * c + j
                    nc.sync.dma_start_transpose(out=v_sb[:, kb, 0:64],
                                                in_=vt_dram[:, kb * 128:(kb + 1) * 128])

            def attn_phase(qc):
                s0 = qc * 512
                sl = slice(s0, s0 + 512)
                nkb = 4 * (qc + 1)
                y0 = pyp.tile([65, 512], F32, tag="ypo", name="y0")
                y1 = pyp.tile([65, 512], F32, tag="ypo", name="y1")
                for kb in range(nkb):
                    stp = pst.tile([128, 1024], F32, tag="st", name="stp")
                    nc.tensor.matmul(stp[:, 0:512], lhsT=kt2_sb[0:64, kb * 128:(kb + 1) * 128],
                                     rhs=qt_sb[0:64, sl], start=True, stop=True,
                                     tile_position=(0, 0))
                    nc.tensor.matmul(stp[:, 512:1024], lhsT=kt2_sb[64:128, kb * 128:(kb + 1) * 128],
                                     rhs=qt_sb[64:128, sl], start=True, stop=True,
                                     tile_position=(64, 0))
                    se = sep.tile([128, 1024], BF16, tag="se", name="se")
                    nc.scalar.activation(se[:], stp[:], AF.Exp, bias=zb128, scale=SCALE)
                    j = kb - 4 * qc
                    if j >= 0:  # diagonal block: zero the non-causal scores (idle Pool engine)
                        se3 = bass.AP(se.tensor, se.offset, [se.ap[0], [512, 2], [1, 512]])
                        nc.gpsimd.affine_select(
                            out=se3, in_=se3, compare_op=mybir.AluOpType.is_ge,
                            fill=0.0, base=-128 * j, pattern=[[0, 2], [1, 512]],
                            channel_multiplier=-1)
                    nc.tensor.matmul(y0[:], lhsT=v_sb[:, kb, 0:65], rhs=se[:, 0:512],
                                     start=(kb == 0), stop=(kb == nkb - 1))
                    nc.tensor.matmul(y1[:], lhsT=v_sb[:, kb, 0:65], rhs=se[:, 512:1024],
                                     start=(kb == 0), stop=(kb == nkb - 1))

                # ---------- softmax normalize -> yt ----------
                s0row = rowp.tile([1, 512], F32, tag="row", name="s0row")
                nc.vector.tensor_copy(s0row[:], y0[64:65, :])
                rps0 = pst.tile([64, 512], F32, tag="st", name="rps0")
                nc.tensor.matmul(rps0[:], lhsT=ones_r[:], rhs=s0row[:], start=True, stop=True)
                nc.vector.reciprocal(rr0[0:64, :], rps0[:])
                nc.vector.tensor_mul(yt_sb[0:64, sl], y0[0:64, :], rr0[0:64, :])
                s1row = rowp.tile([1, 512], F32, tag="row", name="s1row")
                nc.vector.tensor_copy(s1row[:], y1[64:65, :])
                rps1 = pst.tile([64, 512], F32, tag="st", name="rps1")
                nc.tensor.matmul(rps1[:], lhsT=ones_r[:], rhs=s1row[:], start=True, stop=True)
                nc.vector.reciprocal(rr1[0:64, :], rps1[:])
                # h1 write crosses partition quadrants: 32-wide quadrant-aligned moves
                nc.vector.tensor_mul(yt_sb[64:96, sl], y1[0:32, :], rr1[0:32, :])
                nc.vector.tensor_mul(yt_sb[96:128, sl], y1[32:64, :], rr1[32:64, :])

                # ---------- partial projection for this q-chunk ----------
                ot_big = outp.tile([128, 8, 512], F32, tag="ot", name="ot_big")
                for ot in range(8):
                    po = pyp.tile([128, 512], F32, tag="ypo", name="po")
                    nc.tensor.matmul(po[:], lhsT=wproj_sb[:, ot * 128:(ot + 1) * 128],
                                     rhs=yt_sb[:, sl], start=True, stop=True)
                    nc.vector.tensor_copy(ot_big[:, ot, :], po[:])
                out_dst = bass.AP(out_d[:].tensor, s0, [[S, 128], [128 * S, 8], [1, 512]])
                nc.sync.dma_start(out=out_dst, in_=ot_big[:])

            # lookahead-1 pipeline: emit QKV(c+1) before attention(c) so the
            # next chunk's projections/rope overlap the current attention
            qkv_phase(0)
            for c in range(1, NCH):
                qkv_phase(c)
                attn_phase(c - 1)
            attn_phase(NCH - 1)

        if iters > 1:
            with tc.For_i(0, iters, 1) as _i:
                _chunks()
        else:
            _chunks()

    if split:
        split_multi_waits(nc)
    return nc


def split_multi_waits(nc, max_waits=1):
    """walrus's per-instruction sync encoding only fits one sem wait on some
    instruction types (e.g. the matmul LDWEIGHTS struct). Hoist extra waits
    onto same-engine NoOps inserted just before the instruction."""
    nid = [0]
    for fn in nc.m.functions:
        for blk in fn.blocks:
            out = []
            for inst in blk.instructions:
                si = inst.sync_info
                if si is not None and len(si.on_wait) > max_waits:
                    waits = list(si.on_wait)
                    for w in waits[:-max_waits]:
                        nop = mybir.InstNoOp(name=f"waitsplit-{nid[0]}", ins=[], outs=[])
                        nid[0] += 1
                        nop.engine = inst.engine
                        nop.sync_info = mybir.SyncInfo(on_wait=[w], on_update=[])
                        out.append(nop)
                    inst.sync_info = mybir.SyncInfo(on_wait=waits[-max_waits:],
                                                    on_update=list(si.on_update))
                out.append(inst)
            blk.instructions = out


def make_host_inputs(x, Wq, Wk, Wv, Wproj, q_gain, S):
    """Slice/transpose full inputs into per-core in_maps (host-side prep)."""
    xt = np.ascontiguousarray(x.reshape(S, DIM).T).astype(np.float32, copy=False)

    inv_freq = 1.0 / (ROPE_BASE ** (np.arange(0, HD, 2, dtype=np.float32) / HD))
    t = np.arange(S, dtype=np.float32)
    freqs = np.outer(t, inv_freq).astype(np.float32)        # [S, 32]
    cos_t = np.cos(freqs).T.astype(np.float32)              # [32, S]
    sin_t = np.sin(freqs).T.astype(np.float32)
    cos64 = cos_t[PERM64 % 32]                              # [64, S]
    sgn = np.where(PERM64 < 32, 1.0, -1.0).astype(np.float32)[:, None]
    sin64 = sin_t[PERM64 % 32] * sgn
    cos4 = np.ascontiguousarray(np.tile(cos64, (2, 1)))     # [128, S]
    sin4s = np.ascontiguousarray(np.tile(sin64, (2, 1)))

    in_maps = []
    for c in range(N_CORES):
        kv = c // 2
        wq_c = Wq[128 * c:128 * (c + 1), :]                 # [128, 1024]
        wq_c = np.concatenate([wq_c[PERM64], wq_c[64 + PERM64]], axis=0)
        wk_c = Wk[64 * kv:64 * (kv + 1), :][PERM64]         # [64, 1024]
        wv_c = Wv[64 * kv:64 * (kv + 1), :]
        wqkv_t = np.ascontiguousarray(
            np.concatenate([wq_c, wk_c, wv_c], axis=0).T).astype(np.float32, copy=False)
        wproj_t = np.ascontiguousarray(
            Wproj[:, 128 * c:128 * (c + 1)].T).astype(np.float32, copy=False)
        gain = np.ascontiguousarray(np.log(
            np.repeat(q_gain[2 * c:2 * c + 2], 64).reshape(128, 1))).astype(np.float32, copy=False)
        in_maps.append({
            "xt": xt,
            "wqkv_t": wqkv_t,
            "wproj_t": wproj_t,
            "cos4": cos4,
            "sin4s": sin4s,
            "gain": gain,
        })
    return in_maps


_NC_CACHE = {}


def get_nc(S):
    if S not in _NC_CACHE:
        _NC_CACHE[S] = build_nc(S)
    return _NC_CACHE[S]


def kernel(x, Wq, Wk, Wv, Wproj, q_gain, trace=False):
    x = np.asarray(x, dtype=np.float32)
    B, S, D = x.shape
    assert B == 1 and D == DIM
    in_maps = make_host_inputs(
        x, np.asarray(Wq, np.float32), np.asarray(Wk, np.float32),
        np.asarray(Wv, np.float32), np.asarray(Wproj, np.float32),
        np.asarray(q_gain, np.float32), S)

    nc = get_nc(S)
    r = run_bass_kernel_spmd(nc, in_maps, core_ids=list(range(N_CORES)), trace=trace)
    total = np.zeros((DIM, S), dtype=np.float32)
    for c in range(N_CORES):
        total += r.results[c]["out"]
    out = np.ascontiguousarray(total.T).reshape(1, S, DIM)
    if trace:
        kernel._last_results = r
    return out

